# revision 29
# baseline (speedup 1.0000x reference)
"""nn_DirAttention kernel for 8 Trainium2 NeuronCores.

Strategy: data-parallel over batch (B=8, one batch element per core).
Per core, the directional attention

    ah[o,i,j] = sum_k Wc[o,k] * Qh[k,i] * Kh[k,j]   (k = C*L = 4096)

is computed by materialising G[k,(j,i)] = Kh[k,j]*Qh[k,i] per 128-row
k-block on the Vector engine (outer-product broadcast via a
column-duplicated K so every operand presents dense bf16 pairs to the
DVE -> 2x mode), then accumulating ah = Wc' @ G on the PE with even/odd
k-blocks on the two halves of the array.  The h direction runs one
full-width [128,4096] G instruction per k-block (both j-halves, fewer
DVE instructions); the w direction keeps the j-half split so the conv
can share PSUM.  Softmax over the channel (partition) axis uses an ACT
exp with per-partition bias bc; the column sums Z are computed by an
all-ones K=64/M=64 matmul straight into the spare upper partitions of
the same ah PSUM tile (Z replicated across 64 partitions), and a DVE
reciprocal from PSUM produces the 1/Z multiplier tile -- no DRAM
round-trips.  The 3x3 conv runs as shifted accumulating matmuls over
zero-padded SBUF images; BatchNorm scale is folded into the conv
weights on the host and the BN shift rides a constant all-ones image
row through the conv's centre tap.
"""

import sys

for _p in ("/opt/trn_rl_repo",):
    if _p not in sys.path:
        sys.path.append(_p)

import numpy as np
import ml_dtypes

import concourse.bacc as bacc
import concourse.bass as bass
import concourse.mybir as mybir
import concourse.tile as tile
from concourse.bass_utils import run_bass_kernel_spmd

BF16 = mybir.dt.bfloat16
F32 = mybir.dt.float32
B, C, L = 8, 64, 64
N = L * L  # 4096
NKB = 32  # 128-row k-blocks in the C*L contraction
BN_EPS = 1e-5
PAD = L + 2  # 66, padded row stride for the conv images

_CACHE = {}


def _build_nc(debug=False):
    nc = bacc.Bacc(target_bir_lowering=False)

    # ---- DRAM parameters -------------------------------------------------
    xbf = nc.dram_tensor("xbf", [C, N], BF16, kind="ExternalInput")
    wqblk = nc.dram_tensor("wqblk", [64, 64], BF16, kind="ExternalInput")
    wkblk = nc.dram_tensor("wkblk", [64, 64], BF16, kind="ExternalInput")
    wcpt = nc.dram_tensor("wcpt", [128, NKB, 64], BF16, kind="ExternalInput")
    woa = nc.dram_tensor("woa", [128, 9, 64], BF16, kind="ExternalInput")
    wob = nc.dram_tensor("wob", [128, 6, 64], BF16, kind="ExternalInput")
    bqq_d = nc.dram_tensor("bqq", [128, 1], F32, kind="ExternalInput")
    bkk_d = nc.dram_tensor("bkk", [128, 1], F32, kind="ExternalInput")
    bc_d = nc.dram_tensor("bc", [64, 1], F32, kind="ExternalInput")
    bo_d = nc.dram_tensor("bo_eff", [64, 1], F32, kind="ExternalInput")
    dv_d = nc.dram_tensor("d_vec", [64, 1], F32, kind="ExternalInput")
    ident_d = nc.dram_tensor("ident", [128, 64], BF16, kind="ExternalInput")
    y = nc.dram_tensor("y", [C, N], F32, kind="ExternalOutput")
    taps = {}
    if debug:
        for nm, shp, dt in [
            ("t_att", [64, N], BF16), ("t_z", [64, N], BF16),
            ("t_hatt", [64, N], BF16), ("t_watt", [64, N], BF16),
        ]:
            taps[nm] = nc.dram_tensor(nm, shp, dt, kind="ExternalOutput")

    from contextlib import ExitStack
    with tile.TileContext(nc) as tc, ExitStack() as _es:
        consts = _es.enter_context(tc.tile_pool(name="consts", bufs=1))
        qk = _es.enter_context(tc.tile_pool(name="qk", bufs=1))
        work = _es.enter_context(tc.tile_pool(name="work", bufs=2))
        gpool = _es.enter_context(tc.tile_pool(name="gpool", bufs=8))
        dpool = _es.enter_context(tc.tile_pool(name="dscratch", bufs=1, space="DRAM"))

        # ---- constant loads ---------------------------------------------
        # x with a zero tail pad: the projections read the +1 / +64
        # shifted variants straight out of this tile via AP offsets
        x2 = consts.tile([64, N + 64], BF16)
        wq_sb = consts.tile([64, 64], BF16)
        wk_sb = consts.tile([64, 64], BF16)
        wc_sb = consts.tile([128, NKB, 64], BF16)
        woa_sb = consts.tile([128, 9, 64], BF16)
        wob_sb = consts.tile([128, 6, 64], BF16)
        bqq = consts.tile([128, 1], F32)
        bkk = consts.tile([128, 1], F32)
        bcv = consts.tile([64, 1], F32)
        bdv2 = consts.tile([128, 1], F32)
        dvv2 = consts.tile([128, 1], F32)
        ones64 = consts.tile([64, 64], BF16)
        ident_sb = consts.tile([128, 64], BF16)

        nc.vector.memset(x2[:, 4096:4160], 0.0)
        nc.vector.memset(ones64[:], 1.0)
        # critical-path loads first: x halves + Wq/Wk feed the projections
        # x + Wq/Wk gate the projections: spread their issue over the
        # sync/scalar queues, park everything else on the idle tensor queue
        nc.sync.dma_start(out=x2[:, 0:2048], in_=xbf[:, 0:2048])
        nc.scalar.dma_start(out=x2[:, 2048:4096], in_=xbf[:, 2048:4096])
        nc.sync.dma_start(out=wq_sb[:], in_=wqblk[:])
        nc.scalar.dma_start(out=wk_sb[:], in_=wkblk[:])
        nc.scalar.dma_start(out=bqq[:], in_=bqq_d[:])
        nc.scalar.dma_start(out=bkk[:], in_=bkk_d[:])
        nc.sync.dma_start(out=wc_sb[:], in_=wcpt[:])
        nc.sync.dma_start(out=woa_sb[:], in_=woa[:])
        nc.sync.dma_start(out=wob_sb[:], in_=wob[:])
        nc.sync.dma_start(out=bcv[:], in_=bc_d[:])
        nc.sync.dma_start(out=bdv2[0:64], in_=bo_d[:])
        nc.sync.dma_start(out=bdv2[64:128], in_=bo_d[:])
        nc.sync.dma_start(out=ident_sb[:], in_=ident_d[:])
        nc.sync.dma_start(out=dvv2[0:64], in_=dv_d[:])
        nc.sync.dma_start(out=dvv2[64:128], in_=dv_d[:])

        # conv image buffers (zero borders); catB row 64 is an all-ones
        # plane that carries the BN shift through the conv centre tap
        catA = consts.tile([128, PAD * PAD], BF16)  # rows 0-63 x, 64-127 h_att
        catB = consts.tile([128, PAD * PAD], BF16)  # rows 0-63 w_att, 64 ones
        nc.gpsimd.memset(catA[:], 0.0)
        nc.gpsimd.memset(catB[:], 0.0)

        def pad_interior_ap(t, p0, p1, row0=0, nrows=L):
            base = t[p0:p1, :]
            return bass.AP(tensor=base.tensor,
                           offset=base.offset + (row0 + 1) * PAD + 1,
                           ap=[base.ap[0], [PAD, nrows], [1, L]])

        # x part of the conv image
        nc.sync.dma_start(out=pad_interior_ap(catA, 0, 64), in_=x2[:, 0:4096])

        # ---- projections -------------------------------------------------
        # Per direction: Q [128, 32, 64] (block kb = spatial pair, partition
        # = (parity, channel)), Kdup [128, 32, 64, 2] (K duplicated pairs).
        q_t = {d: qk.tile([128, NKB, 64], BF16, tag=f"q{d}", name=f"q_{d}") for d in "hw"}
        kd_t = {d: qk.tile([128, NKB, 64, 2], BF16, tag=f"k{d}", name=f"kd_{d}") for d in "hw"}

        with tc.tile_pool(name="projps", bufs=6, space="PSUM") as pps:
            for d in "hw":
                shift = 1 if d == "h" else 64
                for t8 in range(4):  # 8 g-blocks per psum tile
                    for proj in "qk":
                        wsb = wq_sb if proj == "q" else wk_sb
                        bias = bqq if proj == "q" else bkk
                        ps = pps.tile([128, 8, 64], F32, tag="proj")
                        for half in range(2):  # 4 g per matmul
                            g0 = t8 * 8 + half * 4
                            for sh in range(2):
                                off = sh * shift
                                if d == "h":
                                    rhs = bass.AP(
                                        tensor=x2.tensor,
                                        offset=x2.offset + 2 * g0 + off,
                                        ap=[x2.ap[0], [2, 4], [64, 64]])
                                else:
                                    rhs = bass.AP(
                                        tensor=x2.tensor,
                                        offset=x2.offset + 128 * g0 + off,
                                        ap=[x2.ap[0], [128, 4], [1, 64]])
                                nc.tensor.matmul(
                                    out=ps[sh * 64:(sh + 1) * 64,
                                           half * 4:(half + 1) * 4, :],
                                    lhsT=wsb[:], rhs=rhs,
                                    start=True, stop=True,
                                    skip_group_check=True,
                                    tile_position=(0, sh * 64))
                        if proj == "q":
                            nc.scalar.activation(
                                out=q_t[d][:, t8 * 8:(t8 + 1) * 8, :], in_=ps[:],
                                func=mybir.ActivationFunctionType.Identity,
                                bias=bias[:], scale=1.0)
                        else:
                            for dup in range(2):
                                dst = bass.AP(
                                    tensor=kd_t[d].tensor,
                                    offset=kd_t[d].offset + t8 * 8 * 128 + dup,
                                    ap=[kd_t[d].ap[0], [128, 8], [2, 64]])
                                nc.scalar.activation(
                                    out=dst, in_=ps[:],
                                    func=mybir.ActivationFunctionType.Identity,
                                    bias=bias[:], scale=1.0)

        # ---- attention + softmax + apply + conv --------------------------
        att_t = {d: work.tile([64, L, L], BF16, tag=f"att{d}", bufs=1,
                              name=f"att_{d}") for d in "hw"}
        rzb_t = {d: work.tile([64, L, L], BF16, tag=f"rzb{d}", bufs=1,
                              name=f"rzb_{d}") for d in "hw"}
        rz_t = {d: dpool.tile([64, 64], BF16, tag=f"rz{d}", name=f"rz_{d}")
                for d in "hw"}

        # deferred DVE ops (recips / applies) interleaved into the next
        # G-mul stream so they never stall the DVE queue on latency chains
        pending = []

        def drain_pending(n=1):
            for _ in range(n):
                if pending:
                    pending.pop(0)()

        def emit_fold(ps, jh_cols):
            # fold the odd-half partial into the even-half region via an
            # identity matmul (ACT copy to SBUF, then PE adds it back).
            fold = work.tile([128, 2048], BF16, tag="fold", name="fold", bufs=2)
            nc.scalar.copy(out=fold[64:128, :], in_=ps[64:128, :])
            for ns in range(4):
                nc.tensor.matmul(
                    out=ps[0:64, ns * 512:(ns + 1) * 512],
                    lhsT=ident_sb[64:128, :],
                    rhs=fold[64:128, ns * 512:(ns + 1) * 512],
                    start=False, stop=True,
                    skip_group_check=True,
                    tile_position=(64, 0))

        def emit_exp(ps, d, jh):
            # exp with transposed read: ah[(j,i)] -> att[(i, j)]
            src = bass.AP(tensor=ps.tensor, offset=ps.offset,
                          ap=[[ps.ap[0][0], 64], [1, 64], [64, 32]])
            nc.scalar.activation(
                out=att_t[d][0:64, :, jh * 32:(jh + 1) * 32], in_=src,
                func=mybir.ActivationFunctionType.Exp,
                bias=bcv[:], scale=1.0)

        def emit_z(ps, d, jh):
            # Z column sums as a single row in the spare upper partition of
            # the ah PSUM tile, then DMA straight to DRAM for the lane
            # spread: Z[(i,j)] = sum_c att[c,i,j]
            att = att_t[d]
            for mm in range(4):
                rhs = bass.AP(
                    tensor=att.tensor,
                    offset=att.offset + mm * 16 * 64 + jh * 32,
                    ap=[att.ap[0], [64, 16], [1, 32]])
                nc.tensor.matmul(
                    out=ps[64:65, mm * 512:(mm + 1) * 512],
                    lhsT=ones64[:, 0:1], rhs=rhs,
                    start=True, stop=True,
                    skip_group_check=True,
                    tile_position=(0, 64))
            zrow = work.tile([1, 2048], F32, tag="zrow", bufs=2,
                             name=f"zrow{d}{jh}")
            nc.scalar.copy(out=zrow[:], in_=ps[64:65, 0:2048])
            # spread the 2048 sums over 64 lanes: zs[i, j] = Z[(i, j)]
            zs = work.tile([64, 32], F32, tag="zs", bufs=4, name=f"zs{d}{jh}")
            nc.scalar.dma_start(out=zs[:], in_=zrow[:])
            return zs

        def make_recip(zs, d, jh):
            def go():
                rzs = work.tile([64, 32], BF16, tag="rzs", bufs=4,
                                name=f"rzs{d}{jh}")
                with nc.allow_low_precision(reason="1/Z multiplier in bf16"):
                    nc.vector.reciprocal(out=rzs[:], in_=zs[:])
                nc.scalar.dma_start(out=rz_t[d][:, jh * 32:(jh + 1) * 32],
                                    in_=rzs[:])
                # partition-broadcast 1/Z back as rzb[c, i, j] for all c
                nc.scalar.dma_start(
                    out=bass.AP(tensor=rzb_t[d].tensor,
                                offset=rzb_t[d].offset + jh * 32,
                                ap=[rzb_t[d].ap[0], [64, 64], [1, 32]]),
                    in_=bass.AP(tensor=rz_t[d].tensor,
                                offset=rz_t[d].offset + jh * 32,
                                ap=[[0, 64], [64, 64], [1, 32]]))
            return go

        def make_apply(d):
            # hat = x * att * (1/Z), written straight into the padded conv
            # image; chunked so the conv row-pairs can chase the writes.
            tmp = work.tile([64, N], BF16, tag="tmp", bufs=2, name=f"tmp_{d}")
            cat_dst, cat_p0 = (catA, 64) if d == "h" else (catB, 0)
            steps = [lambda: nc.vector.tensor_mul(
                out=tmp[:], in0=att_t[d][:].rearrange("p a b -> p (a b)"),
                in1=x2[:, 0:4096])]
            for ch in range(4):
                def chunk(ch=ch):
                    sl = slice(ch * 1024, (ch + 1) * 1024)
                    tv = tmp[:, sl].rearrange("p (a b) -> p a b", b=64)
                    rv = rzb_t[d][:, ch * 16:(ch + 1) * 16, :]
                    nc.vector.tensor_mul(
                        out=pad_interior_ap(cat_dst, cat_p0, cat_p0 + 64,
                                            row0=ch * 16, nrows=16),
                        in0=tv, in1=rv)
                steps.append(chunk)
            return steps

        with tc.tile_pool(name="ps", bufs=2, space="PSUM") as aps:
            # ---------------- h direction: full-width G ------------------
            psh = {jh: aps.tile([128, 2048], F32, tag="ps", name=f"psh{jh}")
                   for jh in range(2)}
            q, kd = q_t["h"], kd_t["h"]
            for kb in range(NKB):
                g = gpool.tile([128, 64, 64], BF16, tag="g", name="gh")
                in0 = bass.AP(tensor=kd.tensor, offset=kd.offset + kb * 128,
                              ap=[kd.ap[0], [2, 64], [0, 32], [1, 2]])
                in1 = bass.AP(tensor=q.tensor, offset=q.offset + kb * 64,
                              ap=[q.ap[0], [0, 64], [2, 32], [1, 2]])
                gout = bass.AP(tensor=g.tensor, offset=g.offset,
                               ap=[g.ap[0], [64, 64], [2, 32], [1, 2]])
                nc.vector.tensor_mul(out=gout, in0=in0, in1=in1)
                grhs = g[:].rearrange("p a b -> p (a b)")
                opart = (kb % 2) * 64
                for jh in range(2):
                    for ns in range(4):
                        nc.tensor.matmul(
                            out=psh[jh][opart:opart + 64, ns * 512:(ns + 1) * 512],
                            lhsT=wc_sb[:, kb, :],
                            rhs=grhs[:, jh * 2048 + ns * 512:jh * 2048 + (ns + 1) * 512],
                            start=(kb < 2), stop=(kb >= NKB - 2 and ns == 3),
                            skip_group_check=True,
                            tile_position=(0, opart))
            for jh in range(2):
                emit_fold(psh[jh], jh)
                emit_exp(psh[jh], "h", jh)
                zs = emit_z(psh[jh], "h", jh)
                pending.append(make_recip(zs, "h", jh))
            if debug:
                nc.sync.dma_start(
                    out=taps["t_att"][:],
                    in_=att_t["h"][:].rearrange("p a b -> p (a b)"))

            # ---------------- w direction: j-half split ------------------
            # Both j-halves accumulate single-quadrant (partitions 0-63)
            # so the spare upper partitions of every ps-tagged PSUM buffer
            # stay free for the in-tile Z sums until their recip drains.
            q, kd = q_t["w"], kd_t["w"]
            pending.extend(make_apply("h"))
            for jh in range(2):
                psw = aps.tile([128, 2048], F32, tag="ps", name=f"psw{jh}")
                if jh == 0:
                    cv = aps.tile([128, 2048], F32, tag="ps", name="cv")
                for kbp in range(NKB // 2):
                    grhs = {}
                    gt = gpool.tile([128, 64, 64], BF16, tag="g", name="gw")
                    for half in range(2):
                        kb = kbp * 2 + half
                        in0 = bass.AP(
                            tensor=kd.tensor,
                            offset=kd.offset + kb * 128 + jh * 64,
                            ap=[kd.ap[0], [2, 32], [0, 32], [1, 2]])
                        in1 = bass.AP(
                            tensor=q.tensor, offset=q.offset + kb * 64,
                            ap=[q.ap[0], [0, 32], [2, 32], [1, 2]])
                        gout = bass.AP(
                            tensor=gt.tensor, offset=gt.offset + half * 2048,
                            ap=[gt.ap[0], [64, 32], [2, 32], [1, 2]])
                        nc.vector.tensor_mul(out=gout, in0=in0, in1=in1)
                        grhs[half] = half * 2048
                    # deferred DVE work rides between G muls
                    if kbp in (2, 3, 4, 5, 6, 7, 8):
                        drain_pending(1)
                    gflat = gt[:].rearrange("p a b -> p (a b)")
                    for ns in range(4):
                        for half in range(2):
                            kb = kbp * 2 + half
                            nc.tensor.matmul(
                                out=psw[0:64, ns * 512:(ns + 1) * 512],
                                lhsT=wc_sb[:, kb, :],
                                rhs=gflat[:, grhs[half] + ns * 512:grhs[half] + (ns + 1) * 512],
                                start=(kbp == 0 and half == 0),
                                stop=(kbp == NKB // 2 - 1 and ns == 3),
                                skip_group_check=True,
                                tile_position=(0, 0))
                    ca = {(0, 11): 0, (0, 14): 1, (1, 3): 2, (1, 6): 3}.get(
                        (jh, kbp))
                    if ca is not None:
                        # conv pass A (x + h_att) on the PE wherever the PE
                        # has slack, so its queue drains by the last G block
                        for rp in [ca]:
                            for tap in range(9):
                                dy, dx = tap // 3, tap % 3
                                for half in range(2):
                                    r = rp * 2 + half
                                    off = (r * 8 + dy) * PAD + dx
                                    rhs = bass.AP(tensor=catA.tensor,
                                                  offset=catA.offset + off,
                                                  ap=[catA.ap[0], [PAD, 8], [1, 64]])
                                    nc.tensor.matmul(
                                        out=cv[half * 64:(half + 1) * 64,
                                               rp * 512:(rp + 1) * 512],
                                        lhsT=woa_sb[:, tap, :], rhs=rhs,
                                        start=(tap == 0), stop=False,
                                        skip_group_check=True,
                                        tile_position=(0, half * 64))
                if jh == 0:
                    emit_exp(psw, "w", jh)
                    zs = emit_z(psw, "w", jh)
                    pending.append(make_recip(zs, "w", jh))

            if debug:
                nc.sync.dma_start(
                    out=taps["t_hatt"][:],
                    in_=pad_interior_ap(catA, 64, 128))

            # conv pass B: tap pairs (dy,0)+(dy,1) share one K=128 matmul
            # (catB rows 64-127 hold the w_att image shifted one column);
            # the dx=2 taps ride rows 0-63 with zero upper weights.
            SLOT_OFF = [(0, 0), (1, 0), (2, 0), (0, 2), (1, 2), (2, 2)]

            def conv_tail(rp):
                for slot, (dy, dx) in enumerate(SLOT_OFF):
                    for half in range(2):
                        r = rp * 2 + half
                        off = (r * 8 + dy) * PAD + dx
                        rhs = bass.AP(tensor=catB.tensor, offset=catB.offset + off,
                                      ap=[catB.ap[0], [PAD, 8], [1, 64]])
                        nc.tensor.matmul(out=cv[half * 64:(half + 1) * 64,
                                                rp * 512:(rp + 1) * 512],
                                         lhsT=wob_sb[:, slot, :], rhs=rhs,
                                         start=False, stop=(slot == 5),
                                         skip_group_check=True,
                                         tile_position=(0, half * 64))
                ysb = work.tile([128, 512], F32, tag="ysb", name="ysb")
                nc.vector.tensor_scalar(
                    out=ysb[:], in0=cv[:, rp * 512:(rp + 1) * 512],
                    scalar1=bdv2[:], scalar2=dvv2[:],
                    op0=mybir.AluOpType.add, op1=mybir.AluOpType.max)
                nc.sync.dma_start(out=y[:, (2 * rp) * 512:(2 * rp + 1) * 512],
                                  in_=ysb[0:64, :])
                nc.sync.dma_start(out=y[:, (2 * rp + 1) * 512:(2 * rp + 2) * 512],
                                  in_=ysb[64:128, :])

            # ----- w-jh1 softmax tail, pipelined per 16-row i-chunk -----
            # exp -> Z -> lane-spread -> recip -> broadcast -> apply chase
            # each other across chunks so the DMA latency chains overlap.
            attw = att_t["w"]
            zroww = work.tile([1, 2048], F32, tag="zrow", bufs=2, name="zrow_w1")
            tmpw = work.tile([64, N], BF16, tag="tmp", bufs=2, name="tmp_w")
            zsc_t, rzsc_t = [], []
            for ic in range(4):
                src = bass.AP(tensor=psw.tensor, offset=psw.offset + 16 * ic,
                              ap=[[psw.ap[0][0], 64], [1, 16], [64, 32]])
                nc.scalar.activation(
                    out=attw[0:64, 16 * ic:16 * ic + 16, 32:64], in_=src,
                    func=mybir.ActivationFunctionType.Exp,
                    bias=bcv[:], scale=1.0)
                rhs = bass.AP(tensor=attw.tensor,
                              offset=attw.offset + ic * 16 * 64 + 32,
                              ap=[attw.ap[0], [64, 16], [1, 32]])
                nc.tensor.matmul(
                    out=psw[64:65, ic * 512:(ic + 1) * 512],
                    lhsT=ones64[:, 0:1], rhs=rhs,
                    start=True, stop=True,
                    skip_group_check=True,
                    tile_position=(0, 64))
                nc.vector.tensor_mul(
                    out=tmpw[:, ic * 1024:(ic + 1) * 1024],
                    in0=attw[:, 16 * ic:16 * ic + 16, :].rearrange(
                        "p a b -> p (a b)"),
                    in1=x2[:, ic * 1024:(ic + 1) * 1024])
                if ic == 0:
                    continue
                jc = ic - 1
                nc.scalar.copy(out=zroww[:, jc * 512:(jc + 1) * 512],
                               in_=psw[64:65, jc * 512:(jc + 1) * 512])
                zsc = work.tile([16, 32], F32, tag="zsc", bufs=4,
                                name=f"zsc{jc}")
                nc.sync.dma_start(
                    out=zsc[:], in_=zroww[:, jc * 512:(jc + 1) * 512])
                zsc_t.append(zsc)
            nc.scalar.copy(out=zroww[:, 3 * 512:4 * 512],
                           in_=psw[64:65, 3 * 512:4 * 512])
            zsc = work.tile([16, 32], F32, tag="zsc", bufs=4, name="zsc3")
            nc.sync.dma_start(out=zsc[:], in_=zroww[:, 3 * 512:4 * 512])
            zsc_t.append(zsc)
            for ic in range(4):
                rzsc = work.tile([16, 32], BF16, tag="rzsc", bufs=4,
                                 name=f"rzsc{ic}")
                with nc.allow_low_precision(reason="1/Z multiplier in bf16"):
                    nc.vector.reciprocal(out=rzsc[:], in_=zsc_t[ic][:])
                nc.sync.dma_start(
                    out=rz_t["w"][16 * ic:16 * ic + 16, 32:64], in_=rzsc[:])
                nc.scalar.dma_start(
                    out=bass.AP(tensor=rzb_t["w"].tensor,
                                offset=rzb_t["w"].offset + 16 * ic * 64 + 32,
                                ap=[rzb_t["w"].ap[0], [64, 16], [1, 32]]),
                    in_=bass.AP(tensor=rz_t["w"].tensor,
                                offset=rz_t["w"].offset + 16 * ic * 64 + 32,
                                ap=[[0, 64], [64, 16], [1, 32]]))
            for ic in range(4):
                nc.vector.tensor_mul(
                    out=pad_interior_ap(catB, 0, 64, row0=ic * 16, nrows=16),
                    in0=tmpw[:, ic * 1024:(ic + 1) * 1024].rearrange(
                        "p (a b) -> p a b", b=64),
                    in1=rzb_t["w"][:, ic * 16:(ic + 1) * 16, :])
                s0 = (ic * 16 + 1) * PAD
                s1 = (ic * 16 + 17) * PAD - 1
                nc.sync.dma_start(out=catB[64:128, s0:s1],
                                  in_=catB[0:64, s0 + 1:s1 + 1])
                if ic >= 1:
                    conv_tail(ic - 1)
            conv_tail(3)
            if debug:
                nc.sync.dma_start(
                    out=taps["t_z"][:],
                    in_=att_t["w"][:].rearrange("p a b -> p (a b)"))

            if debug:
                nc.sync.dma_start(
                    out=taps["t_watt"][:],
                    in_=pad_interior_ap(catB, 0, 64))

    nc.finalize()
    return nc


def _host_prep(Wq, bq, Wk, bk, Wc, bc, Wo, bo, gamma, beta, run_mean, run_var):
    bf = ml_dtypes.bfloat16
    wqblk = np.ascontiguousarray(Wq.T)
    wkblk = np.ascontiguousarray(Wk.T)
    # Wc permuted so the contraction index is (spatial, channel)
    wcp = Wc.reshape(C, C, L).transpose(0, 2, 1).reshape(C, C * L)
    wcpt = np.ascontiguousarray(
        wcp.T.reshape(NKB, 128, 64).transpose(1, 0, 2))  # [128, 32, 64]
    inv = gamma / np.sqrt(run_var + BN_EPS)
    wo_eff = Wo * inv[:, None, None, None]
    wot = wo_eff.transpose(1, 2, 3, 0).reshape(3 * C, 9, C)  # [192, 9, 64]
    wot2 = wot[128:192]  # [64, 9, 64] w_att tap weights
    wobp = np.zeros((128, 6, C), np.float32)
    for s, (a, b) in enumerate([(0, 1), (3, 4), (6, 7)]):
        wobp[0:64, s] = wot2[:, a]
        wobp[64:128, s] = wot2[:, b]
    for k, t in enumerate([2, 5, 8]):
        wobp[0:64, 3 + k] = wot2[:, t]
    return {
        "wqblk": wqblk.astype(bf), "wkblk": wkblk.astype(bf),
        "wcpt": wcpt.astype(bf),
        "woa": np.ascontiguousarray(wot[0:128]).astype(bf),
        "wob": wobp.astype(bf),
        "bqq": np.concatenate([bq, bq]).reshape(128, 1).astype(np.float32),
        "bkk": np.concatenate([bk, bk]).reshape(128, 1).astype(np.float32),
        "bc": bc.reshape(64, 1).astype(np.float32),
        "bo_eff": (bo * inv + beta - run_mean * inv).reshape(64, 1).astype(np.float32),
        "d_vec": (beta - run_mean * inv).reshape(64, 1).astype(np.float32),
        "ident": np.concatenate([np.zeros((64, 64), np.float32),
                                 np.eye(64, dtype=np.float32)]).astype(bf),
    }


def kernel(x, Wq, bq, Wk, bk, Wc, bc, Wo, bo, gamma, beta, run_mean, run_var,
           debug=False, trace=False, trace_kwargs=None):
    x = np.asarray(x, np.float32)
    weights = _host_prep(
        np.asarray(Wq, np.float32), np.asarray(bq, np.float32),
        np.asarray(Wk, np.float32), np.asarray(bk, np.float32),
        np.asarray(Wc, np.float32), np.asarray(bc, np.float32),
        np.asarray(Wo, np.float32), np.asarray(bo, np.float32),
        np.asarray(gamma, np.float32), np.asarray(beta, np.float32),
        np.asarray(run_mean, np.float32), np.asarray(run_var, np.float32))
    key = bool(debug)
    if key not in _CACHE:
        _CACHE[key] = _build_nc(debug=debug)
    nc = _CACHE[key]
    bf = ml_dtypes.bfloat16
    in_maps = []
    for b in range(B):
        m = dict(weights)
        m["xbf"] = np.ascontiguousarray(x[b].reshape(C, N)).astype(bf)
        in_maps.append(m)
    kwargs = {}
    if trace:
        kwargs = dict(trace=True, trace_cores=[0], **(trace_kwargs or {}))
    res = run_bass_kernel_spmd(nc, in_maps, core_ids=list(range(B)), **kwargs)
    out = np.stack([res.results[b]["y"].reshape(C, L, L) for b in range(B)])
    if debug or trace:
        return out, res
    return out


# revision 31
# speedup vs baseline: 1.0124x; 1.0124x over previous
"""nn_DirAttention kernel for 8 Trainium2 NeuronCores.

Strategy: data-parallel over batch (B=8, one batch element per core).
Per core, the directional attention

    ah[o,i,j] = sum_k Wc[o,k] * Qh[k,i] * Kh[k,j]   (k = C*L = 4096)

is computed by materialising G[k,(j,i)] = Kh[k,j]*Qh[k,i] per 128-row
k-block on the Vector engine (outer-product broadcast via a
column-duplicated K so every operand presents dense bf16 pairs to the
DVE -> 2x mode), then accumulating ah = Wc' @ G on the PE with even/odd
k-blocks on the two halves of the array.  The h direction runs one
full-width [128,4096] G instruction per k-block (both j-halves, fewer
DVE instructions); the w direction keeps the j-half split so the conv
can share PSUM.  Softmax over the channel (partition) axis uses an ACT
exp with per-partition bias bc; the column sums Z are computed by an
all-ones K=64/M=64 matmul straight into the spare upper partitions of
the same ah PSUM tile (Z replicated across 64 partitions), and a DVE
reciprocal from PSUM produces the 1/Z multiplier tile -- no DRAM
round-trips.  The 3x3 conv runs as shifted accumulating matmuls over
zero-padded SBUF images; BatchNorm scale is folded into the conv
weights on the host and the BN shift rides a constant all-ones image
row through the conv's centre tap.
"""

import sys

for _p in ("/opt/trn_rl_repo",):
    if _p not in sys.path:
        sys.path.append(_p)

import numpy as np
import ml_dtypes

import concourse.bacc as bacc
import concourse.bass as bass
import concourse.mybir as mybir
import concourse.tile as tile
from concourse.bass_utils import run_bass_kernel_spmd

BF16 = mybir.dt.bfloat16
F32 = mybir.dt.float32
B, C, L = 8, 64, 64
N = L * L  # 4096
NKB = 32  # 128-row k-blocks in the C*L contraction
BN_EPS = 1e-5
PAD = L + 2  # 66, padded row stride for the conv images

_CACHE = {}


def _build_nc(debug=False):
    nc = bacc.Bacc(target_bir_lowering=False)

    # ---- DRAM parameters -------------------------------------------------
    xbf = nc.dram_tensor("xbf", [C, N], BF16, kind="ExternalInput")
    wqblk = nc.dram_tensor("wqblk", [64, 64], BF16, kind="ExternalInput")
    wkblk = nc.dram_tensor("wkblk", [64, 64], BF16, kind="ExternalInput")
    wcpt = nc.dram_tensor("wcpt", [128, NKB, 64], BF16, kind="ExternalInput")
    woa = nc.dram_tensor("woa", [128, 9, 64], BF16, kind="ExternalInput")
    wob = nc.dram_tensor("wob", [128, 6, 64], BF16, kind="ExternalInput")
    bqq_d = nc.dram_tensor("bqq", [128, 1], F32, kind="ExternalInput")
    bkk_d = nc.dram_tensor("bkk", [128, 1], F32, kind="ExternalInput")
    bc_d = nc.dram_tensor("bc", [64, 1], F32, kind="ExternalInput")
    bo_d = nc.dram_tensor("bo_eff", [64, 1], F32, kind="ExternalInput")
    dv_d = nc.dram_tensor("d_vec", [64, 1], F32, kind="ExternalInput")
    ident_d = nc.dram_tensor("ident", [128, 64], BF16, kind="ExternalInput")
    y = nc.dram_tensor("y", [C, N], F32, kind="ExternalOutput")
    taps = {}
    if debug:
        for nm, shp, dt in [
            ("t_att", [64, N], BF16), ("t_z", [64, N], BF16),
            ("t_hatt", [64, N], BF16), ("t_watt", [64, N], BF16),
        ]:
            taps[nm] = nc.dram_tensor(nm, shp, dt, kind="ExternalOutput")

    from contextlib import ExitStack
    with tile.TileContext(nc) as tc, ExitStack() as _es:
        consts = _es.enter_context(tc.tile_pool(name="consts", bufs=1))
        qk = _es.enter_context(tc.tile_pool(name="qk", bufs=1))
        work = _es.enter_context(tc.tile_pool(name="work", bufs=2))
        gpool = _es.enter_context(tc.tile_pool(name="gpool", bufs=9))
        dpool = _es.enter_context(tc.tile_pool(name="dscratch", bufs=1, space="DRAM"))

        # ---- constant loads ---------------------------------------------
        # x with a zero tail pad: the projections read the +1 / +64
        # shifted variants straight out of this tile via AP offsets
        x2 = consts.tile([64, N + 64], BF16)
        wq_sb = consts.tile([64, 64], BF16)
        wk_sb = consts.tile([64, 64], BF16)
        wc_sb = consts.tile([128, NKB, 64], BF16)
        woa_sb = consts.tile([128, 9, 64], BF16)
        wob_sb = consts.tile([128, 6, 64], BF16)
        bqq = consts.tile([128, 1], F32)
        bkk = consts.tile([128, 1], F32)
        bcv = consts.tile([64, 1], F32)
        bdv2 = consts.tile([128, 1], F32)
        dvv2 = consts.tile([128, 1], F32)
        ones64 = consts.tile([64, 64], BF16)
        ident_sb = consts.tile([128, 64], BF16)

        nc.vector.memset(x2[:, 4096:4160], 0.0)
        nc.vector.memset(ones64[:], 1.0)
        # critical-path loads first: x halves + Wq/Wk feed the projections
        # x + Wq/Wk gate the projections: spread their issue over the
        # sync/scalar queues, park everything else on the idle tensor queue
        nc.sync.dma_start(out=x2[:, 0:2048], in_=xbf[:, 0:2048])
        nc.scalar.dma_start(out=x2[:, 2048:4096], in_=xbf[:, 2048:4096])
        nc.sync.dma_start(out=wq_sb[:], in_=wqblk[:])
        nc.scalar.dma_start(out=wk_sb[:], in_=wkblk[:])
        nc.scalar.dma_start(out=bqq[:], in_=bqq_d[:])
        nc.scalar.dma_start(out=bkk[:], in_=bkk_d[:])
        nc.sync.dma_start(out=wc_sb[:], in_=wcpt[:])
        nc.sync.dma_start(out=woa_sb[:], in_=woa[:])
        nc.sync.dma_start(out=wob_sb[:], in_=wob[:])
        nc.sync.dma_start(out=bcv[:], in_=bc_d[:])
        nc.sync.dma_start(out=bdv2[0:64], in_=bo_d[:])
        nc.sync.dma_start(out=bdv2[64:128], in_=bo_d[:])
        nc.sync.dma_start(out=ident_sb[:], in_=ident_d[:])
        nc.sync.dma_start(out=dvv2[0:64], in_=dv_d[:])
        nc.sync.dma_start(out=dvv2[64:128], in_=dv_d[:])

        # conv image buffers (zero borders); catB row 64 is an all-ones
        # plane that carries the BN shift through the conv centre tap
        catA = consts.tile([128, PAD * PAD], BF16)  # rows 0-63 x, 64-127 h_att
        catB = consts.tile([128, PAD * PAD], BF16)  # rows 0-63 w_att, 64 ones
        nc.gpsimd.memset(catA[:], 0.0)
        nc.gpsimd.memset(catB[:], 0.0)

        def pad_interior_ap(t, p0, p1, row0=0, nrows=L):
            base = t[p0:p1, :]
            return bass.AP(tensor=base.tensor,
                           offset=base.offset + (row0 + 1) * PAD + 1,
                           ap=[base.ap[0], [PAD, nrows], [1, L]])

        # x part of the conv image
        nc.sync.dma_start(out=pad_interior_ap(catA, 0, 64), in_=x2[:, 0:4096])

        # ---- projections -------------------------------------------------
        # Per direction: Q [128, 32, 64] (block kb = spatial pair, partition
        # = (parity, channel)), Kdup [128, 32, 64, 2] (K duplicated pairs).
        q_t = {d: qk.tile([128, NKB, 64], BF16, tag=f"q{d}", name=f"q_{d}") for d in "hw"}
        kd_t = {d: qk.tile([128, NKB, 64, 2], BF16, tag=f"k{d}", name=f"kd_{d}") for d in "hw"}

        with tc.tile_pool(name="projps", bufs=6, space="PSUM") as pps:
            for d in "hw":
                shift = 1 if d == "h" else 64
                for t8 in range(4):  # 8 g-blocks per psum tile
                    for proj in "qk":
                        wsb = wq_sb if proj == "q" else wk_sb
                        bias = bqq if proj == "q" else bkk
                        ps = pps.tile([128, 8, 64], F32, tag="proj")
                        for half in range(2):  # 4 g per matmul
                            g0 = t8 * 8 + half * 4
                            for sh in range(2):
                                off = sh * shift
                                if d == "h":
                                    rhs = bass.AP(
                                        tensor=x2.tensor,
                                        offset=x2.offset + 2 * g0 + off,
                                        ap=[x2.ap[0], [2, 4], [64, 64]])
                                else:
                                    rhs = bass.AP(
                                        tensor=x2.tensor,
                                        offset=x2.offset + 128 * g0 + off,
                                        ap=[x2.ap[0], [128, 4], [1, 64]])
                                nc.tensor.matmul(
                                    out=ps[sh * 64:(sh + 1) * 64,
                                           half * 4:(half + 1) * 4, :],
                                    lhsT=wsb[:], rhs=rhs,
                                    start=True, stop=True,
                                    skip_group_check=True,
                                    tile_position=(0, sh * 64))
                        if proj == "q":
                            nc.scalar.activation(
                                out=q_t[d][:, t8 * 8:(t8 + 1) * 8, :], in_=ps[:],
                                func=mybir.ActivationFunctionType.Identity,
                                bias=bias[:], scale=1.0)
                        else:
                            for dup in range(2):
                                dst = bass.AP(
                                    tensor=kd_t[d].tensor,
                                    offset=kd_t[d].offset + t8 * 8 * 128 + dup,
                                    ap=[kd_t[d].ap[0], [128, 8], [2, 64]])
                                nc.scalar.activation(
                                    out=dst, in_=ps[:],
                                    func=mybir.ActivationFunctionType.Identity,
                                    bias=bias[:], scale=1.0)

        # ---- attention + softmax + apply + conv --------------------------
        att_t = {d: work.tile([64, L, L], BF16, tag=f"att{d}", bufs=1,
                              name=f"att_{d}") for d in "hw"}
        rzb_t = {d: work.tile([64, L, L], BF16, tag=f"rzb{d}", bufs=1,
                              name=f"rzb_{d}") for d in "hw"}
        rz_t = {d: dpool.tile([64, 64], BF16, tag=f"rz{d}", name=f"rz_{d}")
                for d in "hw"}

        # deferred DVE ops (recips / applies) interleaved into the next
        # G-mul stream so they never stall the DVE queue on latency chains
        pending = []

        def drain_pending(n=1):
            for _ in range(n):
                if pending:
                    pending.pop(0)()

        def emit_fold(ps, jh_cols):
            # fold the odd-half partial into the even-half region via an
            # identity matmul (ACT copy to SBUF, then PE adds it back).
            fold = work.tile([128, 2048], BF16, tag="fold", name="fold", bufs=2)
            nc.scalar.copy(out=fold[64:128, :], in_=ps[64:128, :])
            for ns in range(4):
                nc.tensor.matmul(
                    out=ps[0:64, ns * 512:(ns + 1) * 512],
                    lhsT=ident_sb[64:128, :],
                    rhs=fold[64:128, ns * 512:(ns + 1) * 512],
                    start=False, stop=True,
                    skip_group_check=True,
                    tile_position=(64, 0))

        def emit_exp(ps, d, jh):
            # exp with transposed read: ah[(j,i)] -> att[(i, j)]
            src = bass.AP(tensor=ps.tensor, offset=ps.offset,
                          ap=[[ps.ap[0][0], 64], [1, 64], [64, 32]])
            nc.scalar.activation(
                out=att_t[d][0:64, :, jh * 32:(jh + 1) * 32], in_=src,
                func=mybir.ActivationFunctionType.Exp,
                bias=bcv[:], scale=1.0)

        def emit_z(ps, d, jh):
            # Z column sums as a single row in the spare upper partition of
            # the ah PSUM tile, then DMA straight to DRAM for the lane
            # spread: Z[(i,j)] = sum_c att[c,i,j]
            att = att_t[d]
            for mm in range(4):
                rhs = bass.AP(
                    tensor=att.tensor,
                    offset=att.offset + mm * 16 * 64 + jh * 32,
                    ap=[att.ap[0], [64, 16], [1, 32]])
                nc.tensor.matmul(
                    out=ps[64:65, mm * 512:(mm + 1) * 512],
                    lhsT=ones64[:, 0:1], rhs=rhs,
                    start=True, stop=True,
                    skip_group_check=True,
                    tile_position=(0, 64))
            zrow = work.tile([1, 2048], F32, tag="zrow", bufs=2,
                             name=f"zrow{d}{jh}")
            nc.scalar.copy(out=zrow[:], in_=ps[64:65, 0:2048])
            # spread the 2048 sums over 64 lanes: zs[i, j] = Z[(i, j)]
            zs = work.tile([64, 32], F32, tag="zs", bufs=4, name=f"zs{d}{jh}")
            nc.scalar.dma_start(out=zs[:], in_=zrow[:])
            return zs

        def make_recip(zs, d, jh):
            def go():
                rzs = work.tile([64, 32], BF16, tag="rzs", bufs=4,
                                name=f"rzs{d}{jh}")
                with nc.allow_low_precision(reason="1/Z multiplier in bf16"):
                    nc.vector.reciprocal(out=rzs[:], in_=zs[:])
                nc.scalar.dma_start(out=rz_t[d][:, jh * 32:(jh + 1) * 32],
                                    in_=rzs[:])
                # partition-broadcast 1/Z back as rzb[c, i, j] for all c
                nc.scalar.dma_start(
                    out=bass.AP(tensor=rzb_t[d].tensor,
                                offset=rzb_t[d].offset + jh * 32,
                                ap=[rzb_t[d].ap[0], [64, 64], [1, 32]]),
                    in_=bass.AP(tensor=rz_t[d].tensor,
                                offset=rz_t[d].offset + jh * 32,
                                ap=[[0, 64], [64, 64], [1, 32]]))
            return go

        def make_apply(d):
            # hat = x * att * (1/Z), written straight into the padded conv
            # image; chunked so the conv row-pairs can chase the writes.
            tmp = work.tile([64, N], BF16, tag="tmp", bufs=2, name=f"tmp_{d}")
            cat_dst, cat_p0 = (catA, 64) if d == "h" else (catB, 0)
            steps = [lambda: nc.vector.tensor_mul(
                out=tmp[:], in0=att_t[d][:].rearrange("p a b -> p (a b)"),
                in1=x2[:, 0:4096])]
            for ch in range(4):
                def chunk(ch=ch):
                    sl = slice(ch * 1024, (ch + 1) * 1024)
                    tv = tmp[:, sl].rearrange("p (a b) -> p a b", b=64)
                    rv = rzb_t[d][:, ch * 16:(ch + 1) * 16, :]
                    nc.vector.tensor_mul(
                        out=pad_interior_ap(cat_dst, cat_p0, cat_p0 + 64,
                                            row0=ch * 16, nrows=16),
                        in0=tv, in1=rv)
                steps.append(chunk)
            return steps

        with tc.tile_pool(name="ps", bufs=2, space="PSUM") as aps:
            # ---------------- h direction: full-width G ------------------
            psh = {jh: aps.tile([128, 2048], F32, tag="ps", name=f"psh{jh}")
                   for jh in range(2)}
            q, kd = q_t["h"], kd_t["h"]
            for kb in range(NKB):
                g = gpool.tile([128, 64, 64], BF16, tag="g", name="gh")
                in0 = bass.AP(tensor=kd.tensor, offset=kd.offset + kb * 128,
                              ap=[kd.ap[0], [2, 64], [0, 32], [1, 2]])
                in1 = bass.AP(tensor=q.tensor, offset=q.offset + kb * 64,
                              ap=[q.ap[0], [0, 64], [2, 32], [1, 2]])
                gout = bass.AP(tensor=g.tensor, offset=g.offset,
                               ap=[g.ap[0], [64, 64], [2, 32], [1, 2]])
                nc.vector.tensor_mul(out=gout, in0=in0, in1=in1)
                grhs = g[:].rearrange("p a b -> p (a b)")
                opart = (kb % 2) * 64
                for jh in range(2):
                    for ns in range(4):
                        nc.tensor.matmul(
                            out=psh[jh][opart:opart + 64, ns * 512:(ns + 1) * 512],
                            lhsT=wc_sb[:, kb, :],
                            rhs=grhs[:, jh * 2048 + ns * 512:jh * 2048 + (ns + 1) * 512],
                            start=(kb < 2), stop=(kb >= NKB - 2 and ns == 3),
                            skip_group_check=True,
                            tile_position=(0, opart))
            for jh in range(2):
                emit_fold(psh[jh], jh)
                emit_exp(psh[jh], "h", jh)
                zs = emit_z(psh[jh], "h", jh)
                pending.append(make_recip(zs, "h", jh))
            if debug:
                nc.sync.dma_start(
                    out=taps["t_att"][:],
                    in_=att_t["h"][:].rearrange("p a b -> p (a b)"))

            # ---------------- w direction: j-half split ------------------
            # Both j-halves accumulate single-quadrant (partitions 0-63)
            # so the spare upper partitions of every ps-tagged PSUM buffer
            # stay free for the in-tile Z sums until their recip drains.
            q, kd = q_t["w"], kd_t["w"]
            pending.extend(make_apply("h"))
            for jh in range(2):
                psw = aps.tile([128, 2048], F32, tag="ps", name=f"psw{jh}")
                if jh == 0:
                    cv = aps.tile([128, 2048], F32, tag="ps", name="cv")
                for kbp in range(NKB // 2):
                    grhs = {}
                    gt = gpool.tile([128, 64, 64], BF16, tag="g", name="gw")
                    for half in range(2):
                        kb = kbp * 2 + half
                        in0 = bass.AP(
                            tensor=kd.tensor,
                            offset=kd.offset + kb * 128 + jh * 64,
                            ap=[kd.ap[0], [2, 32], [0, 32], [1, 2]])
                        in1 = bass.AP(
                            tensor=q.tensor, offset=q.offset + kb * 64,
                            ap=[q.ap[0], [0, 32], [2, 32], [1, 2]])
                        gout = bass.AP(
                            tensor=gt.tensor, offset=gt.offset + half * 2048,
                            ap=[gt.ap[0], [64, 32], [2, 32], [1, 2]])
                        nc.vector.tensor_mul(out=gout, in0=in0, in1=in1)
                        grhs[half] = half * 2048
                    # deferred DVE work rides between G muls
                    if kbp in (2, 3, 4, 5, 6, 7, 8):
                        drain_pending(1)
                    gflat = gt[:].rearrange("p a b -> p (a b)")
                    for ns in range(4):
                        for half in range(2):
                            kb = kbp * 2 + half
                            nc.tensor.matmul(
                                out=psw[0:64, ns * 512:(ns + 1) * 512],
                                lhsT=wc_sb[:, kb, :],
                                rhs=gflat[:, grhs[half] + ns * 512:grhs[half] + (ns + 1) * 512],
                                start=(kbp == 0 and half == 0),
                                stop=(kbp == NKB // 2 - 1 and ns == 3),
                                skip_group_check=True,
                                tile_position=(0, 0))
                    ca = {(0, 11): 0, (0, 14): 1, (1, 3): 2, (1, 6): 3}.get(
                        (jh, kbp))
                    if ca is not None:
                        # conv pass A (x + h_att) on the PE wherever the PE
                        # has slack, so its queue drains by the last G block
                        for rp in [ca]:
                            for tap in range(9):
                                dy, dx = tap // 3, tap % 3
                                for half in range(2):
                                    r = rp * 2 + half
                                    off = (r * 8 + dy) * PAD + dx
                                    rhs = bass.AP(tensor=catA.tensor,
                                                  offset=catA.offset + off,
                                                  ap=[catA.ap[0], [PAD, 8], [1, 64]])
                                    nc.tensor.matmul(
                                        out=cv[half * 64:(half + 1) * 64,
                                               rp * 512:(rp + 1) * 512],
                                        lhsT=woa_sb[:, tap, :], rhs=rhs,
                                        start=(tap == 0), stop=False,
                                        skip_group_check=True,
                                        tile_position=(0, half * 64))
                if jh == 0:
                    emit_exp(psw, "w", jh)
                    zs = emit_z(psw, "w", jh)
                    pending.append(make_recip(zs, "w", jh))

            if debug:
                nc.sync.dma_start(
                    out=taps["t_hatt"][:],
                    in_=pad_interior_ap(catA, 64, 128))

            # conv pass B: tap pairs (dy,0)+(dy,1) share one K=128 matmul
            # (catB rows 64-127 hold the w_att image shifted one column);
            # the dx=2 taps ride rows 0-63 with zero upper weights.
            SLOT_OFF = [(0, 0), (1, 0), (2, 0), (0, 2), (1, 2), (2, 2)]

            def conv_tail(rp):
                for slot, (dy, dx) in enumerate(SLOT_OFF):
                    for half in range(2):
                        r = rp * 2 + half
                        off = (r * 8 + dy) * PAD + dx
                        rhs = bass.AP(tensor=catB.tensor, offset=catB.offset + off,
                                      ap=[catB.ap[0], [PAD, 8], [1, 64]])
                        nc.tensor.matmul(out=cv[half * 64:(half + 1) * 64,
                                                rp * 512:(rp + 1) * 512],
                                         lhsT=wob_sb[:, slot, :], rhs=rhs,
                                         start=False, stop=(slot == 5),
                                         skip_group_check=True,
                                         tile_position=(0, half * 64))
                ysb = work.tile([128, 512], F32, tag="ysb", name="ysb")
                nc.vector.tensor_scalar(
                    out=ysb[:], in0=cv[:, rp * 512:(rp + 1) * 512],
                    scalar1=bdv2[:], scalar2=dvv2[:],
                    op0=mybir.AluOpType.add, op1=mybir.AluOpType.max)
                nc.sync.dma_start(out=y[:, (2 * rp) * 512:(2 * rp + 1) * 512],
                                  in_=ysb[0:64, :])
                nc.sync.dma_start(out=y[:, (2 * rp + 1) * 512:(2 * rp + 2) * 512],
                                  in_=ysb[64:128, :])

            # ----- w-jh1 softmax tail, pipelined per 16-row i-chunk -----
            # exp -> Z -> lane-spread -> recip -> broadcast -> apply chase
            # each other across chunks so the DMA latency chains overlap.
            attw = att_t["w"]
            zroww = work.tile([1, 2048], F32, tag="zrow", bufs=2, name="zrow_w1")
            tmpw = work.tile([64, N], BF16, tag="tmp", bufs=2, name="tmp_w")
            zsc_t, rzsc_t = [], []
            for ic in range(4):
                src = bass.AP(tensor=psw.tensor, offset=psw.offset + 16 * ic,
                              ap=[[psw.ap[0][0], 64], [1, 16], [64, 32]])
                nc.scalar.activation(
                    out=attw[0:64, 16 * ic:16 * ic + 16, 32:64], in_=src,
                    func=mybir.ActivationFunctionType.Exp,
                    bias=bcv[:], scale=1.0)
                rhs = bass.AP(tensor=attw.tensor,
                              offset=attw.offset + ic * 16 * 64 + 32,
                              ap=[attw.ap[0], [64, 16], [1, 32]])
                nc.tensor.matmul(
                    out=psw[64:65, ic * 512:(ic + 1) * 512],
                    lhsT=ones64[:, 0:1], rhs=rhs,
                    start=True, stop=True,
                    skip_group_check=True,
                    tile_position=(0, 64))
                nc.vector.tensor_mul(
                    out=tmpw[:, ic * 1024:(ic + 1) * 1024],
                    in0=attw[:, 16 * ic:16 * ic + 16, :].rearrange(
                        "p a b -> p (a b)"),
                    in1=x2[:, ic * 1024:(ic + 1) * 1024])
                if ic == 0:
                    continue
                jc = ic - 1
                nc.scalar.copy(out=zroww[:, jc * 512:(jc + 1) * 512],
                               in_=psw[64:65, jc * 512:(jc + 1) * 512])
                zsc = work.tile([16, 32], F32, tag="zsc", bufs=4,
                                name=f"zsc{jc}")
                nc.sync.dma_start(
                    out=zsc[:], in_=zroww[:, jc * 512:(jc + 1) * 512])
                zsc_t.append(zsc)
            nc.scalar.copy(out=zroww[:, 3 * 512:4 * 512],
                           in_=psw[64:65, 3 * 512:4 * 512])
            zsc = work.tile([16, 32], F32, tag="zsc", bufs=4, name="zsc3")
            nc.sync.dma_start(out=zsc[:], in_=zroww[:, 3 * 512:4 * 512])
            zsc_t.append(zsc)
            for ic in range(4):
                rzsc = work.tile([16, 32], BF16, tag="rzsc", bufs=4,
                                 name=f"rzsc{ic}")
                with nc.allow_low_precision(reason="1/Z multiplier in bf16"):
                    nc.vector.reciprocal(out=rzsc[:], in_=zsc_t[ic][:])
                nc.sync.dma_start(
                    out=rz_t["w"][16 * ic:16 * ic + 16, 32:64], in_=rzsc[:])
                nc.scalar.dma_start(
                    out=bass.AP(tensor=rzb_t["w"].tensor,
                                offset=rzb_t["w"].offset + 16 * ic * 64 + 32,
                                ap=[rzb_t["w"].ap[0], [64, 16], [1, 32]]),
                    in_=bass.AP(tensor=rz_t["w"].tensor,
                                offset=rz_t["w"].offset + 16 * ic * 64 + 32,
                                ap=[[0, 64], [64, 16], [1, 32]]))
            for ic in range(4):
                nc.vector.tensor_mul(
                    out=pad_interior_ap(catB, 0, 64, row0=ic * 16, nrows=16),
                    in0=tmpw[:, ic * 1024:(ic + 1) * 1024].rearrange(
                        "p (a b) -> p a b", b=64),
                    in1=rzb_t["w"][:, ic * 16:(ic + 1) * 16, :])
                s0 = (ic * 16 + 1) * PAD
                s1 = (ic * 16 + 17) * PAD - 1
                nc.sync.dma_start(out=catB[64:128, s0:s1],
                                  in_=catB[0:64, s0 + 1:s1 + 1])
                if ic >= 1:
                    conv_tail(ic - 1)
            conv_tail(3)
            if debug:
                nc.sync.dma_start(
                    out=taps["t_z"][:],
                    in_=att_t["w"][:].rearrange("p a b -> p (a b)"))

            if debug:
                nc.sync.dma_start(
                    out=taps["t_watt"][:],
                    in_=pad_interior_ap(catB, 0, 64))

    nc.finalize()
    return nc


def _host_prep(Wq, bq, Wk, bk, Wc, bc, Wo, bo, gamma, beta, run_mean, run_var):
    bf = ml_dtypes.bfloat16
    wqblk = np.ascontiguousarray(Wq.T)
    wkblk = np.ascontiguousarray(Wk.T)
    # Wc permuted so the contraction index is (spatial, channel)
    wcp = Wc.reshape(C, C, L).transpose(0, 2, 1).reshape(C, C * L)
    wcpt = np.ascontiguousarray(
        wcp.T.reshape(NKB, 128, 64).transpose(1, 0, 2))  # [128, 32, 64]
    inv = gamma / np.sqrt(run_var + BN_EPS)
    wo_eff = Wo * inv[:, None, None, None]
    wot = wo_eff.transpose(1, 2, 3, 0).reshape(3 * C, 9, C)  # [192, 9, 64]
    wot2 = wot[128:192]  # [64, 9, 64] w_att tap weights
    wobp = np.zeros((128, 6, C), np.float32)
    for s, (a, b) in enumerate([(0, 1), (3, 4), (6, 7)]):
        wobp[0:64, s] = wot2[:, a]
        wobp[64:128, s] = wot2[:, b]
    for k, t in enumerate([2, 5, 8]):
        wobp[0:64, 3 + k] = wot2[:, t]
    return {
        "wqblk": wqblk.astype(bf), "wkblk": wkblk.astype(bf),
        "wcpt": wcpt.astype(bf),
        "woa": np.ascontiguousarray(wot[0:128]).astype(bf),
        "wob": wobp.astype(bf),
        "bqq": np.concatenate([bq, bq]).reshape(128, 1).astype(np.float32),
        "bkk": np.concatenate([bk, bk]).reshape(128, 1).astype(np.float32),
        "bc": bc.reshape(64, 1).astype(np.float32),
        "bo_eff": (bo * inv + beta - run_mean * inv).reshape(64, 1).astype(np.float32),
        "d_vec": (beta - run_mean * inv).reshape(64, 1).astype(np.float32),
        "ident": np.concatenate([np.zeros((64, 64), np.float32),
                                 np.eye(64, dtype=np.float32)]).astype(bf),
    }


def kernel(x, Wq, bq, Wk, bk, Wc, bc, Wo, bo, gamma, beta, run_mean, run_var,
           debug=False, trace=False, trace_kwargs=None):
    x = np.asarray(x, np.float32)
    weights = _host_prep(
        np.asarray(Wq, np.float32), np.asarray(bq, np.float32),
        np.asarray(Wk, np.float32), np.asarray(bk, np.float32),
        np.asarray(Wc, np.float32), np.asarray(bc, np.float32),
        np.asarray(Wo, np.float32), np.asarray(bo, np.float32),
        np.asarray(gamma, np.float32), np.asarray(beta, np.float32),
        np.asarray(run_mean, np.float32), np.asarray(run_var, np.float32))
    key = bool(debug)
    if key not in _CACHE:
        _CACHE[key] = _build_nc(debug=debug)
    nc = _CACHE[key]
    bf = ml_dtypes.bfloat16
    in_maps = []
    for b in range(B):
        m = dict(weights)
        m["xbf"] = np.ascontiguousarray(x[b].reshape(C, N)).astype(bf)
        in_maps.append(m)
    kwargs = {}
    if trace:
        kwargs = dict(trace=True, trace_cores=[0], **(trace_kwargs or {}))
    res = run_bass_kernel_spmd(nc, in_maps, core_ids=list(range(B)), **kwargs)
    out = np.stack([res.results[b]["y"].reshape(C, L, L) for b in range(B)])
    if debug or trace:
        return out, res
    return out
